# revision 17
# baseline (speedup 1.0000x reference)
"""Distributed Trainium2 kernel: Gemma-style attention block (B=2,T=2048,H=2048,
NH=16,NKV=4,HD=128) across 8 NeuronCores.

Sharding: batch x head-group. Core c handles batch c//4 with q heads
{4g..4g+3} (g = c%4) and kv head g (GQA groups align exactly).  Activations
are kept feature-major ([d_part, t_free]) so every matmul contracts on the
partition dim.  Softmax is max-free (safe: rmsnorm bounds |scores| <=
sqrt(HD)); denominators and rmsnorm sum-of-squares are computed pre-broadcast
via an all-ones stationary matmul.  The 4 per-batch o_proj partials are summed
on host.

Perf structure (hill-climbed against NTFF traces):
- x pre-tiled host-side so each 512-token block is ONE contiguous 2MB DMA
- constants split: qkv weights (first-MM gate) in sub-DMAs on the ACT ring,
  rope tables next, o_proj weights + causal microblock mask last
- phase1 rmsnorm: Square on ScalarE straight from PSUM; rstd =
  exp(-0.5*ln(ssq/HD)) on ScalarE (DVE reciprocal is 6 cpe - too slow)
- phase2 causal triangle: S^T/exp/den/PV restricted to valid query columns;
  single shared [128,128] upper-tri mask on the diagonal microblock only;
  S^T matmuls software-pipelined DEPTH tiles ahead of the ACT exp
- phase3 o_proj: 2048-wide output rows, PSUM->SBUF copies 3:1 VectorE/ScalarE,
  one 512KB output DMA per 128-token row
"""

import os
import sys

sys.path.insert(0, "/opt/trn_rl_repo")

import numpy as np
import ml_dtypes

import concourse.bass as bass
import concourse.mybir as mybir
import concourse.tile as tile
from concourse.bass_utils import run_bass_kernel_spmd

BF16 = ml_dtypes.bfloat16

B, T, H = 2, 2048, 2048
NH, NKV, HD = 16, 4, 128
THETA = 10000.0
NCORES = 8
GPB = 4                    # head-groups (cores) per batch
QHL = NH // GPB // B * 2   # 4 q heads per core
BT = B * T
NBLK = T // 512            # 4 blocks of 512 tokens per batch
NFT = QHL + 2              # feature tiles per ht: q0..q3, k, v
SCALE = 1.0 / np.sqrt(HD)

LAST_RESULTS = None        # stash for test harness profiling

# packed constants W [128, NCW]: qkv weights (first-MM gate)
NCW = 16 * NFT * 128       # per ht: 6 x 128 feature cols
# packed constants A [128, NCA]: rope tables, q-needed cols first so the
# DMA can be split A1 (q tables+sin+ones) / A2 (k tables) around W chunks
OFF_CQ = 0                 # 2048
OFF_RQ = OFF_CQ + T        # 128
OFF_SIN = OFF_RQ + HD      # 2048
OFF_ONES = OFF_SIN + T     # 128
OFF_CK = OFF_ONES + 128    # 2048
OFF_RK = OFF_CK + T        # 128
NCA = OFF_RK + HD
OFF_A1 = OFF_ONES + 128    # A1 = [0, OFF_A1), A2 = [OFF_A1, NCA)
# packed constants B [128, NCB]: o_proj weights + causal microblock mask
OFF_WO = 0                 # QHL*2048
OFF_TRI = OFF_WO + QHL * H
NCB = OFF_TRI + 128


def _rope_tables(w_q, w_k):
    """rope(w*q) = cosw * q + sin * (R_w @ q) where cosw = cos*(1+w) and
    R_w = rot_half matrix with the +-1 and the (1+w) source weight folded in.
    Returns cosw_q, cosw_k, sin (plain), rotmT_q, rotmT_k (lhsT layout)."""
    inv = 1.0 / (THETA ** (np.arange(0, HD, 2, dtype=np.float64) / HD))  # [64]
    t = np.arange(T, dtype=np.float64)
    fr = np.outer(inv, t)                      # [64, T]
    emb = np.concatenate([fr, fr], 0)          # [HD, T]
    cos, sin = np.cos(emb), np.sin(emb)
    cosws, rotms = [], []
    for w in (w_q, w_k):
        wp = 1.0 + w.astype(np.float64)
        cosws.append((cos * wp[:, None]).astype(BF16))
        R = np.zeros((HD, HD))
        for m in range(64):
            R[m, m + 64] = -wp[m + 64]
        for m in range(64, HD):
            R[m, m - 64] = +wp[m - 64]
        rotms.append(np.ascontiguousarray(R.T).astype(BF16))  # lhsT[k, m] = R[m, k]
    return cosws[0], cosws[1], sin.astype(BF16), rotms[0], rotms[1]


def _legalize_waits(nc):
    """This container's walrus accepts only ONE sync wait per instruction
    (even shipped Tile kernels fail codegen). Split each multi-wait
    instruction into single-wait NOPs on the same engine followed by the
    original holding the last wait — per-engine program order makes this
    exactly equivalent."""
    nid = 0
    for fn in nc.m.functions:
        for blk in fn.blocks:
            out = []
            for inst in blk.instructions:
                si = getattr(inst, "sync_info", None)
                if si is not None and si.on_wait and len(si.on_wait) > 1:
                    waits = list(si.on_wait)
                    ups = list(si.on_update) if si.on_update else []
                    for w in waits[:-1]:
                        nop = mybir.InstNoOp(name=f"swx-{nid}", ins=[], outs=[])
                        nid += 1
                        nop.engine = inst.engine
                        nop.sync_info = mybir.SyncInfo(on_wait=[w], on_update=[])
                        out.append(nop)
                    inst.sync_info = mybir.SyncInfo(
                        on_wait=[waits[-1]], on_update=ups)
                out.append(inst)
            blk.instructions = out
    return nc


def _act_direct(nc, out, in_, func, scale=1.0):
    """Emit InstActivation directly, bypassing the bass wrapper (needed for
    Reciprocal, which the wrapper rejects wholesale; our inputs are positive
    and well-scaled, measured max rel err 1.2e-5)."""
    eng = nc.scalar
    inputs = [eng.lower_ap(in_)]
    for arg in (0.0, scale, 0.0):  # bias, scale, alpha
        inputs.append(mybir.ImmediateValue(dtype=mybir.dt.float32, value=arg))
    return eng.add_instruction(
        mybir.InstActivation(
            name=nc.get_next_instruction_name(),
            func=func, ins=inputs, outs=[eng.lower_ap(out)]))


def _build_graph(cfg=None):
    cfg = {**dict(xtp=4, tmp=3, pacc=2, pden=2, depth=2, warm=40),
           **(cfg or {})}
    nc = bass.Bass()
    f32, bf16 = mybir.dt.float32, mybir.dt.bfloat16

    # x pre-tiled on host (this core's batch): row bi*128+p, col ht*512+c
    xB = nc.dram_tensor("xB", [NBLK * 128, 16 * 512], bf16, kind="ExternalInput")
    constsW = nc.dram_tensor("constsW", [128, NCW], bf16, kind="ExternalInput")
    constsA = nc.dram_tensor("constsA", [128, NCA], bf16, kind="ExternalInput")
    constsB = nc.dram_tensor("constsB", [128, NCB], bf16, kind="ExternalInput")
    out = nc.dram_tensor("out", [T, H], bf16, kind="ExternalOutput")

    with tile.TileContext(nc) as tc:
        with (
            tc.tile_pool(name="singles", bufs=1) as singles,
            tc.tile_pool(name="xtp", bufs=cfg["xtp"]) as xtp,
            tc.tile_pool(name="tmp", bufs=cfg["tmp"]) as tmp,
            tc.tile_pool(name="tmpa", bufs=4) as tmpa,
            tc.tile_pool(name="osbp", bufs=2) as osbp,
            tc.tile_pool(name="psum", bufs=cfg["pacc"], space="PSUM") as pacc,
            tc.tile_pool(name="psden", bufs=cfg["pden"], space="PSUM") as pden,
            tc.tile_pool(name="psmm", bufs=2, space="PSUM") as pmm,
        ):
            # ---- resident constants ----
            constsW_sb = singles.tile([128, NCW], bf16)
            constsA_sb = singles.tile([128, NCA], bf16)
            constsB_sb = singles.tile([128, NCB], bf16)
            # W chunks land just-in-time for their chains; rope tables are
            # interleaved (q-part after W1, k-part after W2) so neither the
            # first chains nor the epilogues wait
            for q6 in range(6):     # sub-DMAs: first accum MMs start sooner
                c0, c1 = q6 * 2048, min((q6 + 1) * 2048, NCW)
                nc.scalar.dma_start(
                    out=constsW_sb[:, c0:c1], in_=constsW[:, c0:c1])
                if q6 == 1:
                    nc.scalar.dma_start(out=constsA_sb[:, :OFF_A1],
                                        in_=constsA[:, :OFF_A1])
                elif q6 == 2:
                    nc.scalar.dma_start(out=constsA_sb[:, OFF_A1:],
                                        in_=constsA[:, OFF_A1:])
            wqkv_sb = constsW_sb
            cq_sb = constsA_sb[:, OFF_CQ:OFF_CQ + T]
            ck_sb = constsA_sb[:, OFF_CK:OFF_CK + T]
            sin_sb = constsA_sb[:, OFF_SIN:OFF_SIN + T]
            rq_sb = constsA_sb[:, OFF_RQ:OFF_RQ + HD]
            rk_sb = constsA_sb[:, OFF_RK:OFF_RK + HD]
            ones_sb = constsA_sb[:, OFF_ONES:OFF_ONES + 128]
            wo_sb = constsB_sb[:, OFF_WO:OFF_WO + QHL * H]
            tri_sb = constsB_sb[:, OFF_TRI:OFF_TRI + 128]

            # ---- activations (feature-major), one batch per core ----
            qT = singles.tile([128, QHL * T], bf16, name="qT", tag="qT")
            kT = singles.tile([128, T], bf16, name="kT", tag="kT")
            vn = singles.tile([128, 16 * 128], bf16, name="vn", tag="vn")
            attnT = singles.tile([128, QHL * T], bf16, name="attnT", tag="attnT")

            ACT_F = mybir.ActivationFunctionType
            FT = NFT * 128          # 768 feature cols per ht in wqkv

            # ---- PE warm-up: HAM releases the clock gate after ~3.4us of
            # sustained activity; dummy matmuls on a zeroed tile during the
            # initial DMA wait mean real MMs run at 2.4GHz from the start ----
            if cfg["warm"] > 0:
                wz = singles.tile([128, 512], bf16, name="warmz")
                nc.vector.memset(wz, 0.0)
                wps = pmm.tile([128, 1024], f32, tag="mm", name="warmps")
                for _ in range(cfg["warm"]):
                    nc.tensor.matmul(wps[:, 0:512], lhsT=wz[:, 0:128], rhs=wz,
                                     start=True, stop=True)

            # ---- x tiles all resident (xtp=4); loads staggered so they don't
            # saturate per-core HBM alongside the 7.3MB weight stream, but
            # always issued ahead of o_proj output DMAs on the sync ring ----
            xt_tiles = []
            for blk in range(NBLK):
                xt_tiles.append(
                    xtp.tile([128, 16 * 512], bf16, tag="xt", name=f"xt{blk}"))

            def x_load(blk):
                if blk == 0:        # split: ht=0 MMs start early
                    for q4 in range(4):
                        nc.sync.dma_start(
                            out=xt_tiles[0][:, q4 * 2048:(q4 + 1) * 2048],
                            in_=xB[0:128, q4 * 2048:(q4 + 1) * 2048])
                else:
                    nc.sync.dma_start(
                        out=xt_tiles[blk],
                        in_=xB[blk * 128:(blk + 1) * 128, :])

            x_load(0)
            x_load(1)

            def p1_chain(blk, dt, xts):
                # q0..q3, k projection accumulation chain (feature-major out)
                ps = pacc.tile([128, 512], f32, tag="acc")
                for ht in range(16):
                    nc.tensor.matmul(
                        ps,
                        lhsT=wqkv_sb[:, dt * 2048 + ht * 128:dt * 2048 + (ht + 1) * 128],
                        rhs=xts[ht], start=(ht == 0), stop=(ht == 15))
                return ps

            def p1_post(blk, dt, ps):
                # rmsnorm + rope epilogue; its PE ops (ssq/rot matmuls) are
                # issued a chain late so ACT's Square/Copy latency is hidden
                t0 = blk * 512
                sq = tmp.tile([128, 512], bf16, tag="sq")
                nc.scalar.activation(out=sq, in_=ps, func=ACT_F.Square)
                traw = tmp.tile([128, 512], bf16, tag="traw")
                nc.scalar.activation(out=traw, in_=ps, func=ACT_F.Copy)
                sr2 = pmm.tile([128, 1024], f32, tag="mm", name="sr2")
                ssq = sr2[:, 0:512]
                nc.tensor.matmul(ssq, lhsT=ones_sb, rhs=sq, start=True, stop=True)
                # rstd = exp(-0.5*ln(ssq/HD)) = 1/sqrt(ssq/HD) on ScalarE
                lssq = tmpa.tile([128, 512], f32, tag="rec")
                nc.scalar.activation(
                    out=lssq, in_=ssq, func=ACT_F.Ln, scale=1.0 / HD)
                rstd = tmp.tile([128, 512], bf16, tag="rstd")
                nc.scalar.activation(
                    out=rstd, in_=lssq, func=ACT_F.Exp, scale=-0.5)
                cos_t, rot_t = (cq_sb, rq_sb) if dt < QHL else (ck_sb, rk_sb)
                t1 = tmp.tile([128, 512], bf16, tag="t1")
                nc.vector.tensor_mul(t1, traw, cos_t[:, t0:t0 + 512])
                rps = sr2[:, 512:1024]
                nc.tensor.matmul(rps, lhsT=rot_t, rhs=traw, start=True, stop=True)
                t2 = tmp.tile([128, 512], bf16, tag="t2")
                nc.vector.tensor_mul(t2, rps, sin_sb[:, t0:t0 + 512])
                nc.vector.tensor_add(out=t1, in0=t1, in1=t2)
                dest = (qT[:, dt * T + t0:dt * T + t0 + 512] if dt < QHL
                        else kT[:, t0:t0 + 512])
                nc.vector.tensor_mul(dest, t1, rstd)

            def phase1_block(blk):
                xt_all = xt_tiles[blk]
                xts = [xt_all[:, ht * 512:(ht + 1) * 512] for ht in range(16)]
                # software pipeline: chain(dt+1) issued before post(dt) so the
                # PE never waits on ACT between accumulation chains
                ps_prev = p1_chain(blk, 0, xts)
                for dt in range(1, QHL + 1):
                    ps = p1_chain(blk, dt, xts)
                    p1_post(blk, dt - 1, ps_prev)
                    ps_prev = ps
                # v projection, natural layout [t_part, d_free]
                vps = pacc.tile([128, 512], f32, tag="acc")
                for c4 in range(4):
                    for ht in range(16):
                        nc.tensor.matmul(
                            vps[:, c4 * 128:(c4 + 1) * 128],
                            lhsT=xts[ht][:, c4 * 128:(c4 + 1) * 128],
                            rhs=wqkv_sb[:, 5 * 2048 + ht * 128:5 * 2048 + (ht + 1) * 128],
                            start=(ht == 0), stop=(ht == 15))
                p1_post(blk, QHL, ps_prev)
                with nc.allow_low_precision(reason="bf16 act copy"):
                    nc.vector.tensor_copy(
                        out=vn[:, blk * 512:(blk + 1) * 512], in_=vps)

            def attn_block(h, j):
                # Software-pipelined in PAIRS of 128-key tiles: each pair
                # shares one 2-bank PSUM tile; off-diagonal pairs get ONE
                # [128,1024] ACT exp (halves the per-op overhead in the
                # ACT-bound attention phases).  Causal restriction: tile
                # i >= 4j only covers query columns >= 128*(i-4j); the
                # boundary microblock gets the shared upper-tri mask.
                DEPTH = cfg["depth"]    # lookahead in pairs
                ntk = 4 * j + 4
                npair = ntk // 2
                aps = pacc.tile([128, 512], f32, tag="acc")
                dps = pden.tile([128, 512], f32, tag="den")
                sps_l, pt_l = [], []

                def c0_of(i):
                    r = i - 4 * j
                    return 128 * r if r > 0 else 0

                def issue_st_pair(p):
                    sps2 = pmm.tile([128, 1024], f32, tag="mm", name="sps2")
                    for half in (0, 1):
                        i = 2 * p + half
                        c0 = c0_of(i)
                        nc.tensor.matmul(
                            sps2[:, half * 512 + c0:(half + 1) * 512],
                            lhsT=kT[:, i * 128:(i + 1) * 128],
                            rhs=qT[:, h * T + j * 512 + c0:h * T + (j + 1) * 512],
                            start=True, stop=True)
                    sps_l.append(sps2)

                def issue_exp_pair(p):
                    i0 = 2 * p
                    diag = (i0 - 4 * j) >= 0
                    pt2 = tmpa.tile([128, 1024], bf16, tag="pt", name="pt")
                    if not diag:
                        nc.scalar.activation(
                            out=pt2, in_=sps_l[p], func=ACT_F.Exp, scale=SCALE)
                    else:
                        # per-half exps: the region before each diagonal c0
                        # was never written in PSUM; don't read it
                        for half in (0, 1):
                            c0 = c0_of(i0 + half)
                            s0 = half * 512 + c0
                            nc.scalar.activation(
                                out=pt2[:, s0:(half + 1) * 512],
                                in_=sps_l[p][:, s0:(half + 1) * 512],
                                func=ACT_F.Exp, scale=SCALE)
                        for half in (0, 1):
                            c0 = c0_of(i0 + half)
                            s0 = half * 512 + c0
                            nc.gpsimd.tensor_mul(
                                pt2[:, s0:s0 + 128], pt2[:, s0:s0 + 128],
                                tri_sb)
                    pt_l.append(pt2)

                for p in range(min(DEPTH, npair)):
                    issue_st_pair(p)
                issue_exp_pair(0)
                for p in range(npair):
                    if p + DEPTH < npair:
                        issue_st_pair(p + DEPTH)
                    if p + 1 < npair:
                        issue_exp_pair(p + 1)
                    for half in (0, 1):
                        i = 2 * p + half
                        c0 = c0_of(i)
                        s0 = half * 512 + c0
                        pt = pt_l[p]
                        nc.tensor.matmul(
                            dps[:, c0:], lhsT=ones_sb,
                            rhs=pt[:, s0:(half + 1) * 512],
                            start=(i == 0), stop=(i == ntk - 1))
                        nc.tensor.matmul(
                            aps[:, c0:], lhsT=vn[:, i * 128:(i + 1) * 128],
                            rhs=pt[:, s0:(half + 1) * 512], start=(i == 0),
                            stop=(i == ntk - 1))
                # recip = exp(-ln(den)) on ScalarE: both funcs live in the
                # resident ACT table (the Reciprocal LUT lives in another
                # table and every swap costs 2x 1.28us ACT_TABLE_LOAD; the
                # DVE iterative divide is 3.3us and serializes the engine).
                # Ln reads dps straight from PSUM, releasing the pden bank
                # immediately; the aps bank is released by a fast DVE copy
                # and the normalization becomes a cheap bf16 2x-mode multiply.
                at = attnT[:, h * T + j * 512:h * T + (j + 1) * 512]
                lden = tmpa.tile([128, 512], mybir.dt.float32, tag="rec")
                nc.scalar.activation(out=lden, in_=dps, func=ACT_F.Ln)
                recip = tmpa.tile([128, 512], bf16, tag="recb")
                nc.scalar.activation(out=recip, in_=lden, func=ACT_F.Exp,
                                     scale=-1.0)
                praw = tmpa.tile([128, 512], bf16, tag="praw")
                with nc.allow_low_precision(reason="bf16 attn, host-summed f32"):
                    nc.vector.tensor_copy(out=praw, in_=aps)
                nc.vector.tensor_mul(at, praw, recip)

            def oproj_row(m):
                # full 2048-wide output row of 128 tokens: 4 accumulations into
                # separate PSUM tiles, copies gathered into one SBUF tile, ONE
                # output DMA (512KB, 4KB/partition contiguous).  The final row
                # pipelines per-512-col chunk DMAs so the kernel tail after the
                # last matmul is one small copy + 128KB DMA.
                split = (m == 15)
                osb = osbp.tile([128, 2048], bf16, tag="osb", name="osb")
                ops2 = None
                for j in range(NBLK):
                    if j % 2 == 0:
                        ops2 = pmm.tile([128, 1024], f32, tag="mm", name="ops2")
                    ops = ops2[:, (j % 2) * 512:(j % 2 + 1) * 512]
                    for hh in range(QHL):
                        nc.tensor.matmul(
                            ops,
                            lhsT=attnT[:, hh * T + m * 128:hh * T + (m + 1) * 128],
                            rhs=wo_sb[:, hh * H + j * 512:hh * H + (j + 1) * 512],
                            start=(hh == 0), stop=(hh == QHL - 1))
                    # all copies on DVE: ScalarE is the binding engine in the
                    # attention phases these rows interleave with
                    with nc.allow_low_precision(reason="bf16 partials, host-summed f32"):
                        nc.vector.tensor_copy(
                            out=osb[:, j * 512:(j + 1) * 512], in_=ops)
                    if split:
                        nc.sync.dma_start(
                            out=out[m * 128:(m + 1) * 128,
                                    j * 512:(j + 1) * 512],
                            in_=osb[:, j * 512:(j + 1) * 512])
                if not split:
                    nc.sync.dma_start(
                        out=out[m * 128:(m + 1) * 128, :], in_=osb)

            # phase interleave: attention for query block j only needs
            # k/v/q blocks <= j; o_proj rows 4j..4j+3 only need attnT of
            # block j (all heads). Spreads ACT/DVE/DMA phases into the
            # PE-dense projection phase.
            phase1_block(0)
            # o_proj weights + mask load deferred past the cold-start
            # transfers, but early enough to beat the first o_proj row
            nc.scalar.dma_start(out=constsB_sb, in_=constsB[:, :])
            x_load(2)
            phase1_block(1)
            x_load(3)
            for h in range(QHL):
                attn_block(h, 0)
            phase1_block(2)
            for m in range(0, 4):
                oproj_row(m)
            for h in range(QHL):
                attn_block(h, 1)
            phase1_block(3)
            for m in range(4, 8):
                oproj_row(m)
            for h in range(QHL):
                attn_block(h, 2)
            for m in range(8, 12):
                oproj_row(m)
            for h in range(QHL):
                attn_block(h, 3)
            for m in range(12, 16):
                oproj_row(m)
    return nc


_GRAPH = None


def kernel(x, Wq, Wk, Wv, Wo, q_norm_w, k_norm_w):
    global _GRAPH, LAST_RESULTS
    x = np.asarray(x, dtype=np.float32)
    Wq = np.asarray(Wq, dtype=np.float32)
    Wk = np.asarray(Wk, dtype=np.float32)
    Wv = np.asarray(Wv, dtype=np.float32)
    Wo = np.asarray(Wo, dtype=np.float32)
    q_norm_w = np.asarray(q_norm_w, dtype=np.float32)
    k_norm_w = np.asarray(k_norm_w, dtype=np.float32)

    xT = np.ascontiguousarray(x.reshape(BT, H).T).astype(BF16)
    # pre-tiled blocks: xBm[bi*128+p, ht*512+c] = xT[ht*128+p, bi*512+c]
    xBm = np.ascontiguousarray(
        xT.reshape(16, 128, B * NBLK, 512).transpose(2, 1, 0, 3)
        .reshape(B * NBLK * 128, 16 * 512))
    cos_q, cos_k, sin_d, rotm_q, rotm_k = _rope_tables(q_norm_w, k_norm_w)
    p = np.arange(128)[:, None]
    f = np.arange(128)[None, :]
    tri = (f >= p).astype(BF16)       # upper-tri incl diagonal

    in_maps = []
    for c in range(NCORES):
        b, g = c // GPB, c % GPB
        w_all = np.concatenate([
            Wq[QHL * HD * g:QHL * HD * (g + 1)],
            Wk[HD * g:HD * (g + 1)],
            Wv[HD * g:HD * (g + 1)]], 0)              # [768, H]
        wqkvT = np.ascontiguousarray(w_all.T).astype(BF16)       # [H, 768]
        woT = np.ascontiguousarray(
            Wo[:, QHL * HD * g:QHL * HD * (g + 1)].T).astype(BF16)  # [QHL*HD, H]
        # dt-major packing: col = dt*2048 + ht*128 + q, so each 512KB
        # weight sub-DMA delivers one full feature tile in consumption order
        cw = np.ascontiguousarray(
            wqkvT.reshape(16, 128, NFT, 128).transpose(1, 2, 0, 3)
            .reshape(128, NCW)).astype(BF16)
        ca = np.zeros((128, NCA), dtype=BF16)
        ca[:, OFF_CQ:OFF_CQ + T] = cos_q
        ca[:, OFF_CK:OFF_CK + T] = cos_k
        ca[:, OFF_SIN:OFF_SIN + T] = sin_d
        ca[:, OFF_RQ:OFF_RQ + HD] = rotm_q
        ca[:, OFF_RK:OFF_RK + HD] = rotm_k
        ca[:, OFF_ONES:OFF_ONES + 128] = 1.0
        cb = np.zeros((128, NCB), dtype=BF16)
        cb[:, OFF_WO:OFF_WO + QHL * H] = (
            woT.reshape(QHL, 128, H).transpose(1, 0, 2).reshape(128, QHL * H))
        cb[:, OFF_TRI:OFF_TRI + 128] = tri
        in_maps.append({
            "xB": xBm[b * NBLK * 128:(b + 1) * NBLK * 128],
            "constsW": cw, "constsA": ca, "constsB": cb})

    if _GRAPH is None:
        import json as _json
        cfg = _json.loads(os.environ.get("ATTN_CFG", "{}")) or None
        _GRAPH = _legalize_waits(_build_graph(cfg=cfg))

    want_trace = bool(int(os.environ.get("ATTN_TRACE", "0")))
    try:
        res = run_bass_kernel_spmd(
            _GRAPH, in_maps, core_ids=list(range(NCORES)), trace=want_trace)
    except ModuleNotFoundError:
        if not want_trace:
            raise
        # axon NTFF profile hook unavailable in this environment
        res = run_bass_kernel_spmd(
            _GRAPH, in_maps, core_ids=list(range(NCORES)), trace=False)
    LAST_RESULTS = res
    acc = np.zeros((BT, H), dtype=np.float32)
    for c, r in enumerate(res.results):
        b = c // GPB
        acc[b * T:(b + 1) * T] += r["out"]
    return acc.reshape(B, T, H)



# revision 18
# speedup vs baseline: 1.2535x; 1.2535x over previous
"""Distributed Trainium2 kernel: Gemma-style attention block (B=2,T=2048,H=2048,
NH=16,NKV=4,HD=128) across 8 NeuronCores.

Sharding: batch x head-group. Core c handles batch c//4 with q heads
{4g..4g+3} (g = c%4) and kv head g (GQA groups align exactly).  Activations
are kept feature-major ([d_part, t_free]) so every matmul contracts on the
partition dim.  Softmax is max-free (safe: rmsnorm bounds |scores| <=
sqrt(HD)); denominators and rmsnorm sum-of-squares are computed pre-broadcast
via an all-ones stationary matmul.  The 4 per-batch o_proj partials are summed
on host.

Perf structure (hill-climbed against NTFF traces):
- x pre-tiled host-side so each 512-token block is ONE contiguous 2MB DMA
- constants split: qkv weights (first-MM gate) in sub-DMAs on the ACT ring,
  rope tables next, o_proj weights + causal microblock mask last
- phase1 rmsnorm: Square on ScalarE straight from PSUM; rstd =
  exp(-0.5*ln(ssq/HD)) on ScalarE (DVE reciprocal is 6 cpe - too slow)
- phase2 causal triangle: S^T/exp/den/PV restricted to valid query columns;
  single shared [128,128] upper-tri mask on the diagonal microblock only;
  S^T matmuls software-pipelined DEPTH tiles ahead of the ACT exp
- phase3 o_proj: 2048-wide output rows, PSUM->SBUF copies 3:1 VectorE/ScalarE,
  one 512KB output DMA per 128-token row
"""

import os
import sys

sys.path.insert(0, "/opt/trn_rl_repo")

import numpy as np
import ml_dtypes

import concourse.bass as bass
import concourse.mybir as mybir
import concourse.tile as tile
from concourse.bass_utils import run_bass_kernel_spmd

BF16 = ml_dtypes.bfloat16

B, T, H = 2, 2048, 2048
NH, NKV, HD = 16, 4, 128
THETA = 10000.0
NCORES = 8
GPB = 4                    # head-groups (cores) per batch
QHL = NH // GPB // B * 2   # 4 q heads per core
BT = B * T
NBLK = T // 512            # 4 blocks of 512 tokens per batch
NFT = QHL + 2              # feature tiles per ht: q0..q3, k, v
SCALE = 1.0 / np.sqrt(HD)

LAST_RESULTS = None        # stash for test harness profiling

# packed constants W [128, NCW]: qkv weights (first-MM gate)
NCW = 16 * NFT * 128       # per ht: 6 x 128 feature cols
# packed constants A [128, NCA]: rope tables, q-needed cols first so the
# DMA can be split A1 (q tables+sin+ones) / A2 (k tables) around W chunks
OFF_CQ = 0                 # 2048
OFF_RQ = OFF_CQ + T        # 128
OFF_SIN = OFF_RQ + HD      # 2048
OFF_ONES = OFF_SIN + T     # 128
OFF_CK = OFF_ONES + 128    # 2048
OFF_RK = OFF_CK + T        # 128
NCA = OFF_RK + HD
OFF_A1 = OFF_ONES + 128    # A1 = [0, OFF_A1), A2 = [OFF_A1, NCA)
# packed constants B [128, NCB]: o_proj weights + causal microblock mask
OFF_WO = 0                 # QHL*2048
OFF_TRI = OFF_WO + QHL * H
NCB = OFF_TRI + 128


def _rope_tables(w_q, w_k):
    """rope(w*q) = cosw * q + sin * (R_w @ q) where cosw = cos*(1+w) and
    R_w = rot_half matrix with the +-1 and the (1+w) source weight folded in.
    Returns cosw_q, cosw_k, sin (plain), rotmT_q, rotmT_k (lhsT layout)."""
    inv = 1.0 / (THETA ** (np.arange(0, HD, 2, dtype=np.float64) / HD))  # [64]
    t = np.arange(T, dtype=np.float64)
    fr = np.outer(inv, t)                      # [64, T]
    emb = np.concatenate([fr, fr], 0)          # [HD, T]
    cos, sin = np.cos(emb), np.sin(emb)
    cosws, rotms = [], []
    for w in (w_q, w_k):
        wp = 1.0 + w.astype(np.float64)
        cosws.append((cos * wp[:, None]).astype(BF16))
        R = np.zeros((HD, HD))
        for m in range(64):
            R[m, m + 64] = -wp[m + 64]
        for m in range(64, HD):
            R[m, m - 64] = +wp[m - 64]
        rotms.append(np.ascontiguousarray(R.T).astype(BF16))  # lhsT[k, m] = R[m, k]
    return cosws[0], cosws[1], sin.astype(BF16), rotms[0], rotms[1]


def _legalize_waits(nc):
    """This container's walrus accepts only ONE sync wait per instruction
    (even shipped Tile kernels fail codegen). Split each multi-wait
    instruction into single-wait NOPs on the same engine followed by the
    original holding the last wait — per-engine program order makes this
    exactly equivalent."""
    nid = 0
    for fn in nc.m.functions:
        for blk in fn.blocks:
            out = []
            for inst in blk.instructions:
                si = getattr(inst, "sync_info", None)
                if si is not None and si.on_wait and len(si.on_wait) > 1:
                    waits = list(si.on_wait)
                    ups = list(si.on_update) if si.on_update else []
                    for w in waits[:-1]:
                        nop = mybir.InstNoOp(name=f"swx-{nid}", ins=[], outs=[])
                        nid += 1
                        nop.engine = inst.engine
                        nop.sync_info = mybir.SyncInfo(on_wait=[w], on_update=[])
                        out.append(nop)
                    inst.sync_info = mybir.SyncInfo(
                        on_wait=[waits[-1]], on_update=ups)
                out.append(inst)
            blk.instructions = out
    return nc


def _act_direct(nc, out, in_, func, scale=1.0):
    """Emit InstActivation directly, bypassing the bass wrapper (needed for
    Reciprocal, which the wrapper rejects wholesale; our inputs are positive
    and well-scaled, measured max rel err 1.2e-5)."""
    eng = nc.scalar
    inputs = [eng.lower_ap(in_)]
    for arg in (0.0, scale, 0.0):  # bias, scale, alpha
        inputs.append(mybir.ImmediateValue(dtype=mybir.dt.float32, value=arg))
    return eng.add_instruction(
        mybir.InstActivation(
            name=nc.get_next_instruction_name(),
            func=func, ins=inputs, outs=[eng.lower_ap(out)]))


def _build_graph(cfg=None):
    cfg = {**dict(xtp=4, tmp=3, pacc=2, pden=2, pmm=4, depth=3, warm=40),
           **(cfg or {})}
    nc = bass.Bass()
    f32, bf16 = mybir.dt.float32, mybir.dt.bfloat16

    # x pre-tiled on host (this core's batch): row bi*128+p, col ht*512+c
    xB = nc.dram_tensor("xB", [NBLK * 128, 16 * 512], bf16, kind="ExternalInput")
    constsW = nc.dram_tensor("constsW", [128, NCW], bf16, kind="ExternalInput")
    constsA = nc.dram_tensor("constsA", [128, NCA], bf16, kind="ExternalInput")
    constsB = nc.dram_tensor("constsB", [128, NCB], bf16, kind="ExternalInput")
    out = nc.dram_tensor("out", [T, H], bf16, kind="ExternalOutput")

    with tile.TileContext(nc) as tc:
        with (
            tc.tile_pool(name="singles", bufs=1) as singles,
            tc.tile_pool(name="xtp", bufs=cfg["xtp"]) as xtp,
            tc.tile_pool(name="tmp", bufs=cfg["tmp"]) as tmp,
            tc.tile_pool(name="tmpa", bufs=4) as tmpa,
            tc.tile_pool(name="osbp", bufs=2) as osbp,
            tc.tile_pool(name="psum", bufs=cfg["pacc"], space="PSUM") as pacc,
            tc.tile_pool(name="psden", bufs=cfg["pden"], space="PSUM") as pden,
            tc.tile_pool(name="psmm", bufs=cfg["pmm"], space="PSUM") as pmm,
        ):
            # ---- resident constants ----
            constsW_sb = singles.tile([128, NCW], bf16)
            constsA_sb = singles.tile([128, NCA], bf16)
            constsB_sb = singles.tile([128, NCB], bf16)
            # W chunks land just-in-time for their chains; rope tables are
            # interleaved (q-part after W1, k-part after W2) so neither the
            # first chains nor the epilogues wait
            for q6 in range(6):     # sub-DMAs: first accum MMs start sooner
                c0, c1 = q6 * 2048, min((q6 + 1) * 2048, NCW)
                nc.scalar.dma_start(
                    out=constsW_sb[:, c0:c1], in_=constsW[:, c0:c1])
                if q6 == 1:
                    nc.scalar.dma_start(out=constsA_sb[:, :OFF_A1],
                                        in_=constsA[:, :OFF_A1])
                elif q6 == 2:
                    nc.scalar.dma_start(out=constsA_sb[:, OFF_A1:],
                                        in_=constsA[:, OFF_A1:])
            wqkv_sb = constsW_sb
            cq_sb = constsA_sb[:, OFF_CQ:OFF_CQ + T]
            ck_sb = constsA_sb[:, OFF_CK:OFF_CK + T]
            sin_sb = constsA_sb[:, OFF_SIN:OFF_SIN + T]
            rq_sb = constsA_sb[:, OFF_RQ:OFF_RQ + HD]
            rk_sb = constsA_sb[:, OFF_RK:OFF_RK + HD]
            ones_sb = constsA_sb[:, OFF_ONES:OFF_ONES + 128]
            wo_sb = constsB_sb[:, OFF_WO:OFF_WO + QHL * H]
            tri_sb = constsB_sb[:, OFF_TRI:OFF_TRI + 128]

            # ---- activations (feature-major), one batch per core ----
            qT = singles.tile([128, QHL * T], bf16, name="qT", tag="qT")
            kT = singles.tile([128, T], bf16, name="kT", tag="kT")
            vn = singles.tile([128, 16 * 128], bf16, name="vn", tag="vn")
            attnT = singles.tile([128, QHL * T], bf16, name="attnT", tag="attnT")

            ACT_F = mybir.ActivationFunctionType
            FT = NFT * 128          # 768 feature cols per ht in wqkv

            # ---- PE warm-up: HAM releases the clock gate after ~3.4us of
            # sustained activity; dummy matmuls on a zeroed tile during the
            # initial DMA wait mean real MMs run at 2.4GHz from the start ----
            if cfg["warm"] > 0:
                wz = singles.tile([128, 512], bf16, name="warmz")
                nc.vector.memset(wz, 0.0)
                wps = pmm.tile([128, 512], f32, tag="mm", name="warmps")
                for _ in range(cfg["warm"]):
                    nc.tensor.matmul(wps, lhsT=wz[:, 0:128], rhs=wz,
                                     start=True, stop=True)

            # ---- x tiles all resident (xtp=4); loads staggered so they don't
            # saturate per-core HBM alongside the 7.3MB weight stream, but
            # always issued ahead of o_proj output DMAs on the sync ring ----
            xt_tiles = []
            for blk in range(NBLK):
                xt_tiles.append(
                    xtp.tile([128, 16 * 512], bf16, tag="xt", name=f"xt{blk}"))

            def x_load(blk):
                if blk == 0:        # split: ht=0 MMs start early
                    for q4 in range(4):
                        nc.sync.dma_start(
                            out=xt_tiles[0][:, q4 * 2048:(q4 + 1) * 2048],
                            in_=xB[0:128, q4 * 2048:(q4 + 1) * 2048])
                else:
                    nc.sync.dma_start(
                        out=xt_tiles[blk],
                        in_=xB[blk * 128:(blk + 1) * 128, :])

            x_load(0)
            x_load(1)

            def p1_chain(blk, dt, xts):
                # q0..q3, k projection accumulation chain (feature-major out)
                ps = pacc.tile([128, 512], f32, tag="acc")
                for ht in range(16):
                    nc.tensor.matmul(
                        ps,
                        lhsT=wqkv_sb[:, dt * 2048 + ht * 128:dt * 2048 + (ht + 1) * 128],
                        rhs=xts[ht], start=(ht == 0), stop=(ht == 15))
                return ps

            def p1_post(blk, dt, ps):
                # rmsnorm + rope epilogue; its PE ops (ssq/rot matmuls) are
                # issued a chain late so ACT's Square/Copy latency is hidden
                t0 = blk * 512
                sq = tmp.tile([128, 512], bf16, tag="sq")
                nc.scalar.activation(out=sq, in_=ps, func=ACT_F.Square)
                traw = tmp.tile([128, 512], bf16, tag="traw")
                nc.scalar.activation(out=traw, in_=ps, func=ACT_F.Copy)
                ssq = pmm.tile([128, 512], f32, tag="mm", name="ssq")
                nc.tensor.matmul(ssq, lhsT=ones_sb, rhs=sq, start=True, stop=True)
                # rstd = exp(-0.5*ln(ssq/HD)) = 1/sqrt(ssq/HD) on ScalarE
                lssq = tmpa.tile([128, 512], f32, tag="rec")
                nc.scalar.activation(
                    out=lssq, in_=ssq, func=ACT_F.Ln, scale=1.0 / HD)
                rstd = tmp.tile([128, 512], bf16, tag="rstd")
                nc.scalar.activation(
                    out=rstd, in_=lssq, func=ACT_F.Exp, scale=-0.5)
                cos_t, rot_t = (cq_sb, rq_sb) if dt < QHL else (ck_sb, rk_sb)
                t1 = tmp.tile([128, 512], bf16, tag="t1")
                nc.vector.tensor_mul(t1, traw, cos_t[:, t0:t0 + 512])
                rps = pmm.tile([128, 512], f32, tag="mm", name="rps")
                nc.tensor.matmul(rps, lhsT=rot_t, rhs=traw, start=True, stop=True)
                t2 = tmp.tile([128, 512], bf16, tag="t2")
                nc.vector.tensor_mul(t2, rps, sin_sb[:, t0:t0 + 512])
                nc.vector.tensor_add(out=t1, in0=t1, in1=t2)
                dest = (qT[:, dt * T + t0:dt * T + t0 + 512] if dt < QHL
                        else kT[:, t0:t0 + 512])
                nc.vector.tensor_mul(dest, t1, rstd)

            def phase1_block(blk):
                xt_all = xt_tiles[blk]
                xts = [xt_all[:, ht * 512:(ht + 1) * 512] for ht in range(16)]
                # software pipeline: chain(dt+1) issued before post(dt) so the
                # PE never waits on ACT between accumulation chains
                ps_prev = p1_chain(blk, 0, xts)
                for dt in range(1, QHL + 1):
                    ps = p1_chain(blk, dt, xts)
                    p1_post(blk, dt - 1, ps_prev)
                    ps_prev = ps
                # v projection, natural layout [t_part, d_free]
                vps = pacc.tile([128, 512], f32, tag="acc")
                for c4 in range(4):
                    for ht in range(16):
                        nc.tensor.matmul(
                            vps[:, c4 * 128:(c4 + 1) * 128],
                            lhsT=xts[ht][:, c4 * 128:(c4 + 1) * 128],
                            rhs=wqkv_sb[:, 5 * 2048 + ht * 128:5 * 2048 + (ht + 1) * 128],
                            start=(ht == 0), stop=(ht == 15))
                p1_post(blk, QHL, ps_prev)
                with nc.allow_low_precision(reason="bf16 act copy"):
                    nc.vector.tensor_copy(
                        out=vn[:, blk * 512:(blk + 1) * 512], in_=vps)

            def attn_block(h, j):
                # Software-pipelined: S^T matmuls issued DEPTH tiles ahead so
                # the PE never stalls on the ACT exp of the current tile.
                # Causal restriction: tile i >= 4j only covers query columns
                # >= 128*(i-4j); the 128-wide boundary microblock gets the
                # shared upper-tri mask.  PSUM tiles stay single-bank
                # [128,512]: 2-bank tiles make every matmul write AP
                # non-contiguous and cost ~20% PE throughput (measured).
                DEPTH = cfg["depth"]
                ntk = 4 * j + 4
                aps = pacc.tile([128, 512], f32, tag="acc")
                dps = pden.tile([128, 512], f32, tag="den")
                sps_l, pt_l, c0_l = [], [], []

                def issue_st(i):
                    r = i - 4 * j
                    c0 = 128 * r if r > 0 else 0
                    sps = pmm.tile([128, 512], f32, tag="mm", name="sps")
                    nc.tensor.matmul(
                        sps[:, c0:], lhsT=kT[:, i * 128:(i + 1) * 128],
                        rhs=qT[:, h * T + j * 512 + c0:h * T + (j + 1) * 512],
                        start=True, stop=True)
                    sps_l.append(sps)
                    c0_l.append(c0)

                def issue_exp(i):
                    r = i - 4 * j
                    c0 = c0_l[i]
                    pt = tmpa.tile([128, 512], bf16, tag="pt", name="pt")
                    nc.scalar.activation(
                        out=pt[:, c0:], in_=sps_l[i][:, c0:],
                        func=ACT_F.Exp, scale=SCALE)
                    if r >= 0:
                        nc.gpsimd.tensor_mul(
                            pt[:, c0:c0 + 128], pt[:, c0:c0 + 128], tri_sb)
                    pt_l.append(pt)

                for i in range(min(DEPTH, ntk)):
                    issue_st(i)
                issue_exp(0)
                for i in range(ntk):
                    if i + DEPTH < ntk:
                        issue_st(i + DEPTH)
                    if i + 1 < ntk:
                        issue_exp(i + 1)
                    c0 = c0_l[i]
                    nc.tensor.matmul(dps[:, c0:], lhsT=ones_sb, rhs=pt_l[i][:, c0:],
                                     start=(i == 0), stop=(i == ntk - 1))
                    nc.tensor.matmul(aps[:, c0:], lhsT=vn[:, i * 128:(i + 1) * 128],
                                     rhs=pt_l[i][:, c0:], start=(i == 0),
                                     stop=(i == ntk - 1))
                # recip = exp(-ln(den)) on ScalarE: both funcs live in the
                # resident ACT table (the Reciprocal LUT lives in another
                # table and every swap costs 2x 1.28us ACT_TABLE_LOAD; the
                # DVE iterative divide is 3.3us and serializes the engine).
                # Ln reads dps straight from PSUM, releasing the pden bank
                # immediately; the aps bank is released by a fast DVE copy
                # and the normalization becomes a cheap bf16 2x-mode multiply.
                at = attnT[:, h * T + j * 512:h * T + (j + 1) * 512]
                lden = tmpa.tile([128, 512], mybir.dt.float32, tag="rec")
                nc.scalar.activation(out=lden, in_=dps, func=ACT_F.Ln)
                recip = tmpa.tile([128, 512], bf16, tag="recb")
                nc.scalar.activation(out=recip, in_=lden, func=ACT_F.Exp,
                                     scale=-1.0)
                praw = tmpa.tile([128, 512], bf16, tag="praw")
                with nc.allow_low_precision(reason="bf16 attn, host-summed f32"):
                    nc.vector.tensor_copy(out=praw, in_=aps)
                nc.vector.tensor_mul(at, praw, recip)

            def oproj_row(m):
                # full 2048-wide output row of 128 tokens: 4 accumulations into
                # separate PSUM tiles, copies gathered into one SBUF tile, ONE
                # output DMA (512KB, 4KB/partition contiguous).  The final row
                # pipelines per-512-col chunk DMAs so the kernel tail after the
                # last matmul is one small copy + 128KB DMA.
                split = (m == 15)
                osb = osbp.tile([128, 2048], bf16, tag="osb", name="osb")
                for j in range(NBLK):
                    ops = pmm.tile([128, 512], f32, tag="mm", name="ops")
                    for hh in range(QHL):
                        nc.tensor.matmul(
                            ops,
                            lhsT=attnT[:, hh * T + m * 128:hh * T + (m + 1) * 128],
                            rhs=wo_sb[:, hh * H + j * 512:hh * H + (j + 1) * 512],
                            start=(hh == 0), stop=(hh == QHL - 1))
                    # all copies on DVE: ScalarE is the binding engine in the
                    # attention phases these rows interleave with
                    with nc.allow_low_precision(reason="bf16 partials, host-summed f32"):
                        nc.vector.tensor_copy(
                            out=osb[:, j * 512:(j + 1) * 512], in_=ops)
                    if split:
                        nc.sync.dma_start(
                            out=out[m * 128:(m + 1) * 128,
                                    j * 512:(j + 1) * 512],
                            in_=osb[:, j * 512:(j + 1) * 512])
                if not split:
                    nc.sync.dma_start(
                        out=out[m * 128:(m + 1) * 128, :], in_=osb)

            # phase interleave: attention for query block j only needs
            # k/v/q blocks <= j; o_proj rows 4j..4j+3 only need attnT of
            # block j (all heads). Spreads ACT/DVE/DMA phases into the
            # PE-dense projection phase.
            phase1_block(0)
            # o_proj weights + mask load deferred past the cold-start
            # transfers, but early enough to beat the first o_proj row
            nc.scalar.dma_start(out=constsB_sb, in_=constsB[:, :])
            x_load(2)
            phase1_block(1)
            x_load(3)
            for h in range(QHL):
                attn_block(h, 0)
            phase1_block(2)
            for m in range(0, 4):
                oproj_row(m)
            for h in range(QHL):
                attn_block(h, 1)
            phase1_block(3)
            for m in range(4, 8):
                oproj_row(m)
            for h in range(QHL):
                attn_block(h, 2)
            for m in range(8, 12):
                oproj_row(m)
            for h in range(QHL):
                attn_block(h, 3)
            for m in range(12, 16):
                oproj_row(m)
    return nc


_GRAPH = None


def kernel(x, Wq, Wk, Wv, Wo, q_norm_w, k_norm_w):
    global _GRAPH, LAST_RESULTS
    x = np.asarray(x, dtype=np.float32)
    Wq = np.asarray(Wq, dtype=np.float32)
    Wk = np.asarray(Wk, dtype=np.float32)
    Wv = np.asarray(Wv, dtype=np.float32)
    Wo = np.asarray(Wo, dtype=np.float32)
    q_norm_w = np.asarray(q_norm_w, dtype=np.float32)
    k_norm_w = np.asarray(k_norm_w, dtype=np.float32)

    xT = np.ascontiguousarray(x.reshape(BT, H).T).astype(BF16)
    # pre-tiled blocks: xBm[bi*128+p, ht*512+c] = xT[ht*128+p, bi*512+c]
    xBm = np.ascontiguousarray(
        xT.reshape(16, 128, B * NBLK, 512).transpose(2, 1, 0, 3)
        .reshape(B * NBLK * 128, 16 * 512))
    cos_q, cos_k, sin_d, rotm_q, rotm_k = _rope_tables(q_norm_w, k_norm_w)
    p = np.arange(128)[:, None]
    f = np.arange(128)[None, :]
    tri = (f >= p).astype(BF16)       # upper-tri incl diagonal

    in_maps = []
    for c in range(NCORES):
        b, g = c // GPB, c % GPB
        w_all = np.concatenate([
            Wq[QHL * HD * g:QHL * HD * (g + 1)],
            Wk[HD * g:HD * (g + 1)],
            Wv[HD * g:HD * (g + 1)]], 0)              # [768, H]
        wqkvT = np.ascontiguousarray(w_all.T).astype(BF16)       # [H, 768]
        woT = np.ascontiguousarray(
            Wo[:, QHL * HD * g:QHL * HD * (g + 1)].T).astype(BF16)  # [QHL*HD, H]
        # dt-major packing: col = dt*2048 + ht*128 + q, so each 512KB
        # weight sub-DMA delivers one full feature tile in consumption order
        cw = np.ascontiguousarray(
            wqkvT.reshape(16, 128, NFT, 128).transpose(1, 2, 0, 3)
            .reshape(128, NCW)).astype(BF16)
        ca = np.zeros((128, NCA), dtype=BF16)
        ca[:, OFF_CQ:OFF_CQ + T] = cos_q
        ca[:, OFF_CK:OFF_CK + T] = cos_k
        ca[:, OFF_SIN:OFF_SIN + T] = sin_d
        ca[:, OFF_RQ:OFF_RQ + HD] = rotm_q
        ca[:, OFF_RK:OFF_RK + HD] = rotm_k
        ca[:, OFF_ONES:OFF_ONES + 128] = 1.0
        cb = np.zeros((128, NCB), dtype=BF16)
        cb[:, OFF_WO:OFF_WO + QHL * H] = (
            woT.reshape(QHL, 128, H).transpose(1, 0, 2).reshape(128, QHL * H))
        cb[:, OFF_TRI:OFF_TRI + 128] = tri
        in_maps.append({
            "xB": xBm[b * NBLK * 128:(b + 1) * NBLK * 128],
            "constsW": cw, "constsA": ca, "constsB": cb})

    if _GRAPH is None:
        import json as _json
        cfg = _json.loads(os.environ.get("ATTN_CFG", "{}")) or None
        _GRAPH = _legalize_waits(_build_graph(cfg=cfg))

    want_trace = bool(int(os.environ.get("ATTN_TRACE", "0")))
    try:
        res = run_bass_kernel_spmd(
            _GRAPH, in_maps, core_ids=list(range(NCORES)), trace=want_trace)
    except ModuleNotFoundError:
        if not want_trace:
            raise
        # axon NTFF profile hook unavailable in this environment
        res = run_bass_kernel_spmd(
            _GRAPH, in_maps, core_ids=list(range(NCORES)), trace=False)
    LAST_RESULTS = res
    acc = np.zeros((BT, H), dtype=np.float32)
    for c, r in enumerate(res.results):
        b = c // GPB
        acc[b * T:(b + 1) * T] += r["out"]
    return acc.reshape(B, T, H)



# revision 19
# speedup vs baseline: 1.2891x; 1.0284x over previous
"""Distributed Trainium2 kernel: Gemma-style attention block (B=2,T=2048,H=2048,
NH=16,NKV=4,HD=128) across 8 NeuronCores.

Sharding: batch x head-group. Core c handles batch c//4 with q heads
{4g..4g+3} (g = c%4) and kv head g (GQA groups align exactly).  Activations
are kept feature-major ([d_part, t_free]) so every matmul contracts on the
partition dim.  Softmax is max-free (safe: rmsnorm bounds |scores| <=
sqrt(HD)); denominators and rmsnorm sum-of-squares are computed pre-broadcast
via an all-ones stationary matmul.  The 4 per-batch o_proj partials are summed
on host.

Perf structure (hill-climbed against NTFF traces):
- x pre-tiled host-side so each 512-token block is ONE contiguous 2MB DMA
- constants split: qkv weights (first-MM gate) in sub-DMAs on the ACT ring,
  rope tables next, o_proj weights + causal microblock mask last
- phase1 rmsnorm: Square on ScalarE straight from PSUM; rstd =
  exp(-0.5*ln(ssq/HD)) on ScalarE (DVE reciprocal is 6 cpe - too slow)
- phase2 causal triangle: S^T/exp/den/PV restricted to valid query columns;
  single shared [128,128] upper-tri mask on the diagonal microblock only;
  S^T matmuls software-pipelined DEPTH tiles ahead of the ACT exp
- phase3 o_proj: 2048-wide output rows, PSUM->SBUF copies 3:1 VectorE/ScalarE,
  one 512KB output DMA per 128-token row
"""

import os
import sys

sys.path.insert(0, "/opt/trn_rl_repo")

import numpy as np
import ml_dtypes

import concourse.bass as bass
import concourse.mybir as mybir
import concourse.tile as tile
from concourse.bass_utils import run_bass_kernel_spmd

BF16 = ml_dtypes.bfloat16

B, T, H = 2, 2048, 2048
NH, NKV, HD = 16, 4, 128
THETA = 10000.0
NCORES = 8
GPB = 4                    # head-groups (cores) per batch
QHL = NH // GPB // B * 2   # 4 q heads per core
BT = B * T
NBLK = T // 512            # 4 blocks of 512 tokens per batch
NFT = QHL + 2              # feature tiles per ht: q0..q3, k, v
SCALE = 1.0 / np.sqrt(HD)

LAST_RESULTS = None        # stash for test harness profiling

# packed constants W [128, NCW]: qkv weights (first-MM gate)
NCW = 16 * NFT * 128       # per ht: 6 x 128 feature cols
# packed constants A [128, NCA]: rope tables, q-needed cols first so the
# DMA can be split A1 (q tables+sin+ones) / A2 (k tables) around W chunks
OFF_CQ = 0                 # 2048
OFF_RQ = OFF_CQ + T        # 128
OFF_SIN = OFF_RQ + HD      # 2048
OFF_ONES = OFF_SIN + T     # 128
OFF_CK = OFF_ONES + 128    # 2048
OFF_RK = OFF_CK + T        # 128
NCA = OFF_RK + HD
OFF_A1 = OFF_ONES + 128    # A1 = [0, OFF_A1), A2 = [OFF_A1, NCA)
# packed constants B [128, NCB]: o_proj weights + causal microblock mask
OFF_WO = 0                 # QHL*2048
OFF_TRI = OFF_WO + QHL * H
NCB = OFF_TRI + 128


def _rope_tables(w_q, w_k):
    """rope(w*q) = cosw * q + sin * (R_w @ q) where cosw = cos*(1+w) and
    R_w = rot_half matrix with the +-1 and the (1+w) source weight folded in.
    Returns cosw_q, cosw_k, sin (plain), rotmT_q, rotmT_k (lhsT layout)."""
    inv = 1.0 / (THETA ** (np.arange(0, HD, 2, dtype=np.float64) / HD))  # [64]
    t = np.arange(T, dtype=np.float64)
    fr = np.outer(inv, t)                      # [64, T]
    emb = np.concatenate([fr, fr], 0)          # [HD, T]
    cos, sin = np.cos(emb), np.sin(emb)
    cosws, rotms = [], []
    for w in (w_q, w_k):
        wp = 1.0 + w.astype(np.float64)
        cosws.append((cos * wp[:, None]).astype(BF16))
        R = np.zeros((HD, HD))
        for m in range(64):
            R[m, m + 64] = -wp[m + 64]
        for m in range(64, HD):
            R[m, m - 64] = +wp[m - 64]
        rotms.append(np.ascontiguousarray(R.T).astype(BF16))  # lhsT[k, m] = R[m, k]
    return cosws[0], cosws[1], sin.astype(BF16), rotms[0], rotms[1]


def _legalize_waits(nc):
    """This container's walrus accepts only ONE sync wait per instruction
    (even shipped Tile kernels fail codegen). Split each multi-wait
    instruction into single-wait NOPs on the same engine followed by the
    original holding the last wait — per-engine program order makes this
    exactly equivalent."""
    nid = 0
    for fn in nc.m.functions:
        for blk in fn.blocks:
            out = []
            for inst in blk.instructions:
                si = getattr(inst, "sync_info", None)
                if si is not None and si.on_wait and len(si.on_wait) > 1:
                    waits = list(si.on_wait)
                    ups = list(si.on_update) if si.on_update else []
                    for w in waits[:-1]:
                        nop = mybir.InstNoOp(name=f"swx-{nid}", ins=[], outs=[])
                        nid += 1
                        nop.engine = inst.engine
                        nop.sync_info = mybir.SyncInfo(on_wait=[w], on_update=[])
                        out.append(nop)
                    inst.sync_info = mybir.SyncInfo(
                        on_wait=[waits[-1]], on_update=ups)
                out.append(inst)
            blk.instructions = out
    return nc


def _act_direct(nc, out, in_, func, scale=1.0):
    """Emit InstActivation directly, bypassing the bass wrapper (needed for
    Reciprocal, which the wrapper rejects wholesale; our inputs are positive
    and well-scaled, measured max rel err 1.2e-5)."""
    eng = nc.scalar
    inputs = [eng.lower_ap(in_)]
    for arg in (0.0, scale, 0.0):  # bias, scale, alpha
        inputs.append(mybir.ImmediateValue(dtype=mybir.dt.float32, value=arg))
    return eng.add_instruction(
        mybir.InstActivation(
            name=nc.get_next_instruction_name(),
            func=func, ins=inputs, outs=[eng.lower_ap(out)]))


def _build_graph(cfg=None):
    cfg = {**dict(xtp=4, tmp=3, pacc=2, pden=2, pmm=4, depth=3, warm=32),
           **(cfg or {})}
    nc = bass.Bass()
    f32, bf16 = mybir.dt.float32, mybir.dt.bfloat16

    # x pre-tiled on host (this core's batch): row bi*128+p, col ht*512+c
    xB = nc.dram_tensor("xB", [NBLK * 128, 16 * 512], bf16, kind="ExternalInput")
    constsW = nc.dram_tensor("constsW", [128, NCW], bf16, kind="ExternalInput")
    constsA = nc.dram_tensor("constsA", [128, NCA], bf16, kind="ExternalInput")
    constsB = nc.dram_tensor("constsB", [128, NCB], bf16, kind="ExternalInput")
    out = nc.dram_tensor("out", [T, H], bf16, kind="ExternalOutput")

    with tile.TileContext(nc) as tc:
        with (
            tc.tile_pool(name="singles", bufs=1) as singles,
            tc.tile_pool(name="xtp", bufs=cfg["xtp"]) as xtp,
            tc.tile_pool(name="tmp", bufs=cfg["tmp"]) as tmp,
            tc.tile_pool(name="tmpa", bufs=4) as tmpa,
            tc.tile_pool(name="osbp", bufs=2) as osbp,
            tc.tile_pool(name="psum", bufs=cfg["pacc"], space="PSUM") as pacc,
            tc.tile_pool(name="psden", bufs=cfg["pden"], space="PSUM") as pden,
            tc.tile_pool(name="psmm", bufs=cfg["pmm"], space="PSUM") as pmm,
        ):
            # ---- resident constants ----
            constsW_sb = singles.tile([128, NCW], bf16)
            constsA_sb = singles.tile([128, NCA], bf16)
            constsB_sb = singles.tile([128, NCB], bf16)
            # weight stream split across BOTH DMA rings so warm chains never
            # outrun a single ~150GB/s queue: scalar ring carries W0,W1,A1
            # (q rope tables),W2,A2; the sync ring carries W3..W5 between x
            # block 0 and block 1
            def w_chunk(q6):
                c0, c1 = q6 * 2048, min((q6 + 1) * 2048, NCW)
                return constsW_sb[:, c0:c1], constsW[:, c0:c1]

            for q6 in range(3):
                o, i = w_chunk(q6)
                nc.scalar.dma_start(out=o, in_=i)
                if q6 == 1:
                    nc.scalar.dma_start(out=constsA_sb[:, :OFF_A1],
                                        in_=constsA[:, :OFF_A1])
                elif q6 == 2:
                    nc.scalar.dma_start(out=constsA_sb[:, OFF_A1:],
                                        in_=constsA[:, OFF_A1:])
            wqkv_sb = constsW_sb
            cq_sb = constsA_sb[:, OFF_CQ:OFF_CQ + T]
            ck_sb = constsA_sb[:, OFF_CK:OFF_CK + T]
            sin_sb = constsA_sb[:, OFF_SIN:OFF_SIN + T]
            rq_sb = constsA_sb[:, OFF_RQ:OFF_RQ + HD]
            rk_sb = constsA_sb[:, OFF_RK:OFF_RK + HD]
            ones_sb = constsA_sb[:, OFF_ONES:OFF_ONES + 128]
            wo_sb = constsB_sb[:, OFF_WO:OFF_WO + QHL * H]
            tri_sb = constsB_sb[:, OFF_TRI:OFF_TRI + 128]

            # ---- activations (feature-major), one batch per core ----
            qT = singles.tile([128, QHL * T], bf16, name="qT", tag="qT")
            kT = singles.tile([128, T], bf16, name="kT", tag="kT")
            vn = singles.tile([128, 16 * 128], bf16, name="vn", tag="vn")
            attnT = singles.tile([128, QHL * T], bf16, name="attnT", tag="attnT")

            ACT_F = mybir.ActivationFunctionType
            FT = NFT * 128          # 768 feature cols per ht in wqkv

            # ---- PE warm-up: HAM releases the clock gate after ~3.4us of
            # sustained activity; dummy matmuls on a zeroed tile during the
            # initial DMA wait mean real MMs run at 2.4GHz from the start ----
            if cfg["warm"] > 0:
                wz = singles.tile([128, 512], bf16, name="warmz")
                nc.vector.memset(wz, 0.0)
                wps = pmm.tile([128, 512], f32, tag="mm", name="warmps")
                for _ in range(cfg["warm"]):
                    nc.tensor.matmul(wps, lhsT=wz[:, 0:128], rhs=wz,
                                     start=True, stop=True)

            # ---- x tiles all resident (xtp=4); loads staggered so they don't
            # saturate per-core HBM alongside the 7.3MB weight stream, but
            # always issued ahead of o_proj output DMAs on the sync ring ----
            xt_tiles = []
            for blk in range(NBLK):
                xt_tiles.append(
                    xtp.tile([128, 16 * 512], bf16, tag="xt", name=f"xt{blk}"))

            def x_load(blk):
                if blk == 0:        # fine split: chains follow the DMA frontier
                    for q8 in range(8):
                        nc.sync.dma_start(
                            out=xt_tiles[0][:, q8 * 1024:(q8 + 1) * 1024],
                            in_=xB[0:128, q8 * 1024:(q8 + 1) * 1024])
                else:
                    nc.sync.dma_start(
                        out=xt_tiles[blk],
                        in_=xB[blk * 128:(blk + 1) * 128, :])

            x_load(0)
            for q6 in range(3, 6):
                o, i = w_chunk(q6)
                nc.sync.dma_start(out=o, in_=i)
            x_load(1)

            def p1_chain(blk, dt, xts):
                # q0..q3, k projection accumulation chain (feature-major out)
                ps = pacc.tile([128, 512], f32, tag="acc")
                for ht in range(16):
                    nc.tensor.matmul(
                        ps,
                        lhsT=wqkv_sb[:, dt * 2048 + ht * 128:dt * 2048 + (ht + 1) * 128],
                        rhs=xts[ht], start=(ht == 0), stop=(ht == 15))
                return ps

            def p1_post(blk, dt, ps):
                # rmsnorm + rope epilogue; its PE ops (ssq/rot matmuls) are
                # issued a chain late so ACT's Square/Copy latency is hidden
                t0 = blk * 512
                sq = tmp.tile([128, 512], bf16, tag="sq")
                nc.scalar.activation(out=sq, in_=ps, func=ACT_F.Square)
                traw = tmp.tile([128, 512], bf16, tag="traw")
                nc.scalar.activation(out=traw, in_=ps, func=ACT_F.Copy)
                ssq = pmm.tile([128, 512], f32, tag="mm", name="ssq")
                nc.tensor.matmul(ssq, lhsT=ones_sb, rhs=sq, start=True, stop=True)
                # rstd = exp(-0.5*ln(ssq/HD)) = 1/sqrt(ssq/HD) on ScalarE
                lssq = tmpa.tile([128, 512], f32, tag="rec")
                nc.scalar.activation(
                    out=lssq, in_=ssq, func=ACT_F.Ln, scale=1.0 / HD)
                rstd = tmp.tile([128, 512], bf16, tag="rstd")
                nc.scalar.activation(
                    out=rstd, in_=lssq, func=ACT_F.Exp, scale=-0.5)
                cos_t, rot_t = (cq_sb, rq_sb) if dt < QHL else (ck_sb, rk_sb)
                t1 = tmp.tile([128, 512], bf16, tag="t1")
                nc.vector.tensor_mul(t1, traw, cos_t[:, t0:t0 + 512])
                rps = pmm.tile([128, 512], f32, tag="mm", name="rps")
                nc.tensor.matmul(rps, lhsT=rot_t, rhs=traw, start=True, stop=True)
                t2 = tmp.tile([128, 512], bf16, tag="t2")
                nc.vector.tensor_mul(t2, rps, sin_sb[:, t0:t0 + 512])
                nc.vector.tensor_add(out=t1, in0=t1, in1=t2)
                dest = (qT[:, dt * T + t0:dt * T + t0 + 512] if dt < QHL
                        else kT[:, t0:t0 + 512])
                nc.vector.tensor_mul(dest, t1, rstd)

            def phase1_block(blk):
                xt_all = xt_tiles[blk]
                xts = [xt_all[:, ht * 512:(ht + 1) * 512] for ht in range(16)]
                # software pipeline: chain(dt+1) issued before post(dt) so the
                # PE never waits on ACT between accumulation chains
                ps_prev = p1_chain(blk, 0, xts)
                for dt in range(1, QHL + 1):
                    ps = p1_chain(blk, dt, xts)
                    p1_post(blk, dt - 1, ps_prev)
                    ps_prev = ps
                # v projection, natural layout [t_part, d_free]
                vps = pacc.tile([128, 512], f32, tag="acc")
                for c4 in range(4):
                    for ht in range(16):
                        nc.tensor.matmul(
                            vps[:, c4 * 128:(c4 + 1) * 128],
                            lhsT=xts[ht][:, c4 * 128:(c4 + 1) * 128],
                            rhs=wqkv_sb[:, 5 * 2048 + ht * 128:5 * 2048 + (ht + 1) * 128],
                            start=(ht == 0), stop=(ht == 15))
                p1_post(blk, QHL, ps_prev)
                with nc.allow_low_precision(reason="bf16 act copy"):
                    nc.vector.tensor_copy(
                        out=vn[:, blk * 512:(blk + 1) * 512], in_=vps)

            def attn_block(h, j):
                # Software-pipelined: S^T matmuls issued DEPTH tiles ahead so
                # the PE never stalls on the ACT exp of the current tile.
                # Causal restriction: tile i >= 4j only covers query columns
                # >= 128*(i-4j); the 128-wide boundary microblock gets the
                # shared upper-tri mask.  PSUM tiles stay single-bank
                # [128,512]: 2-bank tiles make every matmul write AP
                # non-contiguous and cost ~20% PE throughput (measured).
                DEPTH = cfg["depth"]
                ntk = 4 * j + 4
                aps = pacc.tile([128, 512], f32, tag="acc")
                dps = pden.tile([128, 512], f32, tag="den")
                sps_l, pt_l, c0_l = [], [], []

                def issue_st(i):
                    r = i - 4 * j
                    c0 = 128 * r if r > 0 else 0
                    sps = pmm.tile([128, 512], f32, tag="mm", name="sps")
                    nc.tensor.matmul(
                        sps[:, c0:], lhsT=kT[:, i * 128:(i + 1) * 128],
                        rhs=qT[:, h * T + j * 512 + c0:h * T + (j + 1) * 512],
                        start=True, stop=True)
                    sps_l.append(sps)
                    c0_l.append(c0)

                def issue_exp(i):
                    r = i - 4 * j
                    c0 = c0_l[i]
                    pt = tmpa.tile([128, 512], bf16, tag="pt", name="pt")
                    nc.scalar.activation(
                        out=pt[:, c0:], in_=sps_l[i][:, c0:],
                        func=ACT_F.Exp, scale=SCALE)
                    if r >= 0:
                        nc.gpsimd.tensor_mul(
                            pt[:, c0:c0 + 128], pt[:, c0:c0 + 128], tri_sb)
                    pt_l.append(pt)

                for i in range(min(DEPTH, ntk)):
                    issue_st(i)
                issue_exp(0)
                for i in range(ntk):
                    if i + DEPTH < ntk:
                        issue_st(i + DEPTH)
                    if i + 1 < ntk:
                        issue_exp(i + 1)
                    c0 = c0_l[i]
                    nc.tensor.matmul(dps[:, c0:], lhsT=ones_sb, rhs=pt_l[i][:, c0:],
                                     start=(i == 0), stop=(i == ntk - 1))
                    nc.tensor.matmul(aps[:, c0:], lhsT=vn[:, i * 128:(i + 1) * 128],
                                     rhs=pt_l[i][:, c0:], start=(i == 0),
                                     stop=(i == ntk - 1))
                # recip = exp(-ln(den)) on ScalarE: both funcs live in the
                # resident ACT table (the Reciprocal LUT lives in another
                # table and every swap costs 2x 1.28us ACT_TABLE_LOAD; the
                # DVE iterative divide is 3.3us and serializes the engine).
                # Ln reads dps straight from PSUM, releasing the pden bank
                # immediately; the aps bank is released by a fast DVE copy
                # and the normalization becomes a cheap bf16 2x-mode multiply.
                at = attnT[:, h * T + j * 512:h * T + (j + 1) * 512]
                lden = tmpa.tile([128, 512], mybir.dt.float32, tag="rec")
                nc.scalar.activation(out=lden, in_=dps, func=ACT_F.Ln)
                recip = tmpa.tile([128, 512], bf16, tag="recb")
                nc.scalar.activation(out=recip, in_=lden, func=ACT_F.Exp,
                                     scale=-1.0)
                praw = tmpa.tile([128, 512], bf16, tag="praw")
                with nc.allow_low_precision(reason="bf16 attn, host-summed f32"):
                    nc.vector.tensor_copy(out=praw, in_=aps)
                nc.vector.tensor_mul(at, praw, recip)

            def oproj_row(m):
                # full 2048-wide output row of 128 tokens: 4 accumulations into
                # separate PSUM tiles, copies gathered into one SBUF tile, ONE
                # output DMA (512KB, 4KB/partition contiguous).  The final row
                # pipelines per-512-col chunk DMAs so the kernel tail after the
                # last matmul is one small copy + 128KB DMA.
                split = (m >= 14)
                osb = osbp.tile([128, 2048], bf16, tag="osb", name="osb")
                for j in range(NBLK):
                    ops = pmm.tile([128, 512], f32, tag="mm", name="ops")
                    for hh in range(QHL):
                        nc.tensor.matmul(
                            ops,
                            lhsT=attnT[:, hh * T + m * 128:hh * T + (m + 1) * 128],
                            rhs=wo_sb[:, hh * H + j * 512:hh * H + (j + 1) * 512],
                            start=(hh == 0), stop=(hh == QHL - 1))
                    # all copies on DVE: ScalarE is the binding engine in the
                    # attention phases these rows interleave with
                    with nc.allow_low_precision(reason="bf16 partials, host-summed f32"):
                        nc.vector.tensor_copy(
                            out=osb[:, j * 512:(j + 1) * 512], in_=ops)
                    if split:
                        nc.sync.dma_start(
                            out=out[m * 128:(m + 1) * 128,
                                    j * 512:(j + 1) * 512],
                            in_=osb[:, j * 512:(j + 1) * 512])
                if not split:
                    nc.sync.dma_start(
                        out=out[m * 128:(m + 1) * 128, :], in_=osb)

            # phase interleave: attention for query block j only needs
            # k/v/q blocks <= j; o_proj rows 4j..4j+3 only need attnT of
            # block j (all heads). Spreads ACT/DVE/DMA phases into the
            # PE-dense projection phase.
            phase1_block(0)
            # o_proj weights + mask load deferred past the cold-start
            # transfers, but early enough to beat the first o_proj row
            nc.scalar.dma_start(out=constsB_sb, in_=constsB[:, :])
            x_load(2)
            phase1_block(1)
            x_load(3)
            for h in range(QHL):
                attn_block(h, 0)
            phase1_block(2)
            for m in range(0, 4):
                oproj_row(m)
            for h in range(QHL):
                attn_block(h, 1)
            phase1_block(3)
            for m in range(4, 8):
                oproj_row(m)
            for h in range(QHL):
                attn_block(h, 2)
            for m in range(8, 12):
                oproj_row(m)
            for h in range(QHL):
                attn_block(h, 3)
            for m in range(12, 16):
                oproj_row(m)
    return nc


_GRAPH = None


def kernel(x, Wq, Wk, Wv, Wo, q_norm_w, k_norm_w):
    global _GRAPH, LAST_RESULTS
    x = np.asarray(x, dtype=np.float32)
    Wq = np.asarray(Wq, dtype=np.float32)
    Wk = np.asarray(Wk, dtype=np.float32)
    Wv = np.asarray(Wv, dtype=np.float32)
    Wo = np.asarray(Wo, dtype=np.float32)
    q_norm_w = np.asarray(q_norm_w, dtype=np.float32)
    k_norm_w = np.asarray(k_norm_w, dtype=np.float32)

    xT = np.ascontiguousarray(x.reshape(BT, H).T).astype(BF16)
    # pre-tiled blocks: xBm[bi*128+p, ht*512+c] = xT[ht*128+p, bi*512+c]
    xBm = np.ascontiguousarray(
        xT.reshape(16, 128, B * NBLK, 512).transpose(2, 1, 0, 3)
        .reshape(B * NBLK * 128, 16 * 512))
    cos_q, cos_k, sin_d, rotm_q, rotm_k = _rope_tables(q_norm_w, k_norm_w)
    p = np.arange(128)[:, None]
    f = np.arange(128)[None, :]
    tri = (f >= p).astype(BF16)       # upper-tri incl diagonal

    in_maps = []
    for c in range(NCORES):
        b, g = c // GPB, c % GPB
        w_all = np.concatenate([
            Wq[QHL * HD * g:QHL * HD * (g + 1)],
            Wk[HD * g:HD * (g + 1)],
            Wv[HD * g:HD * (g + 1)]], 0)              # [768, H]
        wqkvT = np.ascontiguousarray(w_all.T).astype(BF16)       # [H, 768]
        woT = np.ascontiguousarray(
            Wo[:, QHL * HD * g:QHL * HD * (g + 1)].T).astype(BF16)  # [QHL*HD, H]
        # dt-major packing: col = dt*2048 + ht*128 + q, so each 512KB
        # weight sub-DMA delivers one full feature tile in consumption order
        cw = np.ascontiguousarray(
            wqkvT.reshape(16, 128, NFT, 128).transpose(1, 2, 0, 3)
            .reshape(128, NCW)).astype(BF16)
        ca = np.zeros((128, NCA), dtype=BF16)
        ca[:, OFF_CQ:OFF_CQ + T] = cos_q
        ca[:, OFF_CK:OFF_CK + T] = cos_k
        ca[:, OFF_SIN:OFF_SIN + T] = sin_d
        ca[:, OFF_RQ:OFF_RQ + HD] = rotm_q
        ca[:, OFF_RK:OFF_RK + HD] = rotm_k
        ca[:, OFF_ONES:OFF_ONES + 128] = 1.0
        cb = np.zeros((128, NCB), dtype=BF16)
        cb[:, OFF_WO:OFF_WO + QHL * H] = (
            woT.reshape(QHL, 128, H).transpose(1, 0, 2).reshape(128, QHL * H))
        cb[:, OFF_TRI:OFF_TRI + 128] = tri
        in_maps.append({
            "xB": xBm[b * NBLK * 128:(b + 1) * NBLK * 128],
            "constsW": cw, "constsA": ca, "constsB": cb})

    if _GRAPH is None:
        import json as _json
        cfg = _json.loads(os.environ.get("ATTN_CFG", "{}")) or None
        _GRAPH = _legalize_waits(_build_graph(cfg=cfg))

    want_trace = bool(int(os.environ.get("ATTN_TRACE", "0")))
    try:
        res = run_bass_kernel_spmd(
            _GRAPH, in_maps, core_ids=list(range(NCORES)), trace=want_trace)
    except ModuleNotFoundError:
        if not want_trace:
            raise
        # axon NTFF profile hook unavailable in this environment
        res = run_bass_kernel_spmd(
            _GRAPH, in_maps, core_ids=list(range(NCORES)), trace=False)
    LAST_RESULTS = res
    acc = np.zeros((BT, H), dtype=np.float32)
    for c, r in enumerate(res.results):
        b = c // GPB
        acc[b * T:(b + 1) * T] += r["out"]
    return acc.reshape(B, T, H)

